# revision 30
# baseline (speedup 1.0000x reference)
"""GQA + sliding-window attention (B=2, S=2048, E=2048, HQ=16, HKV=4, D=128, W=512).

Sharding: 8 cores = 2 batches x 4 KV-head groups (tensor parallel).
Each core computes its batch's full sequence for one KV head + its 4 Q heads,
plus the (row-sharded) output projection partial; the host sums the 4 partials
per batch (the "all-reduce" done host-side) and adds bo.

v2 layout/schedule (vs v1):
  - k-outer projections accumulate K^T/Q^T/V^T into [128, S] PSUM tiles
    (4 banks each, 2 live), so matmuls start as soon as each xT k-tile's
    DMA lands instead of after the whole 14 MB input load.
  - V computed as V^T then turned into natural [s,d] via 16 PE transposes.
  - scores per k-tile into a double-buffered [128, 640] PSUM tile
    (512-col + 128-col matmuls, bank aligned), masked (DVE) + exp'd
    (ScalarE) without blocking the next k-tile's score matmul.
  - PV runs qi-major one tile behind exp, into a [128, 256] PSUM bank:
    cols 0:128 accumulate V^T E contributions, cols 128:256 accumulate
    the row-sum via an all-ones [k,128] stationary, which lands the
    softmax denominator replicated across all 128 partitions -- so
    normalization is just DVE reciprocal + multiply, no broadcast DMA.
  - output projection interleaved into the last head's loop; out is bf16.
"""

import os

import numpy as np
import ml_dtypes

import concourse.bass as bass
import concourse.mybir as mybir
import concourse.tile as tile
from concourse.tile import add_dep_helper
from concourse.bass_utils import run_bass_kernel_spmd

B, S, E = 2, 2048, 2048
HQ, HKV, D = 16, 4, 128
WINDOW = 512
ROPE_BASE = 10000.0
N_CORES = 8
GROUP = HQ // HKV          # 4 Q heads per KV head
HD_Q = GROUP * D           # 512
ST = S // 128              # 16 sequence tiles
KTILES = E // 128          # 16 contraction tiles over E
WT = WINDOW // 128         # 4 -> window spans WT+1 = 5 q-tiles
NEG = -30000.0

f32 = mybir.dt.float32
bf16 = mybir.dt.bfloat16


def _split_sync_waits(nc, max_waits=1):
    """walrus in this container rejects instructions with more than one
    sync-wait; split extras onto preceding same-engine NoOps."""
    for fn in nc.m.functions:
        for blk in fn.blocks:
            new_insts = []
            for inst in blk.instructions:
                si = getattr(inst, "sync_info", None)
                if si is not None and len(si.on_wait) > max_waits:
                    waits = list(si.on_wait)
                    head, tail = waits[:-max_waits], waits[-max_waits:]
                    for i in range(0, len(head), max_waits):
                        nop = mybir.InstNoOp(
                            name=f"splitwait-{nc.next_id()}",
                            ins=[], outs=[],
                            sync_info=mybir.SyncInfo(
                                on_wait=head[i:i + max_waits], on_update=[]),
                            bass_nofuse=True,
                        )
                        nop.engine = inst.engine
                        new_insts.append(nop)
                    inst.sync_info = mybir.SyncInfo(
                        on_wait=tail, on_update=list(si.on_update))
                new_insts.append(inst)
            blk.instructions[:] = new_insts


def _scalar_recip(nc, out, in_):
    """ScalarE Reciprocal activation. bass bans it for precision-sensitive
    uses (~1e-5 rel err); that is far inside this kernel's error budget and
    it frees the (saturated) DVE from 64 slow InstReciprocals."""
    sc = nc.scalar
    inputs = [sc.lower_ap(in_)]
    for arg in (0.0, 1.0, 0.0):  # bias, scale, alpha immediates
        inputs.append(mybir.ImmediateValue(dtype=mybir.dt.float32, value=arg))
    return sc.add_instruction(mybir.InstActivation(
        name=sc.bass.get_next_instruction_name(),
        func=mybir.ActivationFunctionType.Reciprocal,
        ins=inputs, outs=[sc.lower_ap(out)]))


def build_kernel(has_bias):
    nc = bass.Bass("TRN2", target_bir_lowering=False, debug=False,
                   num_devices=N_CORES)
    Exp = mybir.ActivationFunctionType.Exp

    xT = nc.dram_tensor("xT", [E, S], bf16, kind="ExternalInput").ap()
    wqT = nc.dram_tensor("wqT", [E, HD_Q], bf16, kind="ExternalInput").ap()
    wkT = nc.dram_tensor("wkT", [E, D], bf16, kind="ExternalInput").ap()
    wvT = nc.dram_tensor("wvT", [E, D], bf16, kind="ExternalInput").ap()
    woT = nc.dram_tensor("woT", [HD_Q, E], bf16, kind="ExternalInput").ap()
    cosT = nc.dram_tensor("cosT", [D, S], bf16, kind="ExternalInput").ap()
    sinT = nc.dram_tensor("sinT", [D, S], bf16, kind="ExternalInput").ap()
    if has_bias:
        bqr = nc.dram_tensor("bqr", [1, HD_Q], bf16, kind="ExternalInput").ap()
        bkr = nc.dram_tensor("bkr", [1, D], bf16, kind="ExternalInput").ap()
        bvr = nc.dram_tensor("bvr", [1, D], bf16, kind="ExternalInput").ap()
    out = nc.dram_tensor("out", [S, E], bf16, kind="ExternalOutput").ap()

    with tile.TileContext(nc) as tc:
        with tc.tile_pool(name="singles", bufs=1) as singles, \
             tc.tile_pool(name="upool", bufs=6) as upool, \
             tc.tile_pool(name="epool", bufs=7) as epool, \
             tc.tile_pool(name="rbpool", bufs=3) as rbpool, \
             tc.tile_pool(name="ostage", bufs=4) as ostage:

            # ---- resident tensors ----
            xt = singles.tile([128, KTILES, S], bf16)
            wq = singles.tile([128, KTILES, HD_Q], bf16)
            wk = singles.tile([128, KTILES, D], bf16)
            wv = singles.tile([128, KTILES, D], bf16)
            wo = singles.tile([128, GROUP, E], bf16)
            cost = singles.tile([128, S], bf16)
            sint = singles.tile([128, S], bf16)
            qt = singles.tile([128, GROUP, S], bf16)
            kt = singles.tile([128, S], bf16)
            vtsb = singles.tile([128, S], bf16)
            vv = singles.tile([128, ST, D], bf16)
            ot = singles.tile([128, GROUP * ST, D], bf16)
            mt_diag = singles.tile([128, 128], bf16)
            mt_off4 = singles.tile([128, 128], bf16)
            ones128 = singles.tile([128, 128], bf16)
            ident = singles.tile([128, 128], bf16)

            # Input loads. Descriptor ISSUE on one queue is ~650ns each, so
            # bulk weights go as single rearranged descriptors, and the two
            # tensors P1 needs at k=0 (wk, wq head 0) issue on ScalarE's DMA
            # queue so they don't queue behind the xT tile stream on sync's.
            def ptd(t):  # [(t p) d] dram layout -> [p t d]
                return t.rearrange("(t p) d -> p t d", p=128)

            nc.scalar.dma_start(out=wk[:], in_=ptd(wkT))
            nc.scalar.dma_start(out=wq[:, :, 0:128], in_=ptd(wqT[:, 0:128]))
            for t in range(KTILES):
                # alternate queues: one queue's ~650ns/descriptor issue rate
                # can't keep the P1 k-loop fed on its own
                eng = nc.sync if t % 2 == 0 else nc.gpsimd
                eng.dma_start(out=xt[:, t, :], in_=xT[t * 128:(t + 1) * 128, :])
            nc.sync.dma_start(out=sint[:], in_=sinT)
            nc.sync.dma_start(out=cost[:], in_=cosT)
            nc.sync.dma_start(out=wq[:, :, 128:256], in_=ptd(wqT[:, 128:256]))
            nc.sync.dma_start(out=wv[:], in_=ptd(wvT))
            nc.sync.dma_start(out=wq[:, :, 256:384], in_=ptd(wqT[:, 256:384]))
            nc.sync.dma_start(out=wq[:, :, 384:512], in_=ptd(wqT[:, 384:512]))
            nc.sync.dma_start(out=wo[:], in_=woT.rearrange("(h p) e -> p h e", p=128))
            bq_t = bk_t = bv_t = onesrow = None
            if has_bias:
                bq_t = singles.tile([1, HD_Q], bf16)
                bk_t = singles.tile([1, D], bf16)
                bv_t = singles.tile([1, D], bf16)
                onesrow = singles.tile([1, 512], bf16)
                nc.sync.dma_start(out=bq_t[:], in_=bqr)
                nc.sync.dma_start(out=bk_t[:], in_=bkr)
                nc.sync.dma_start(out=bv_t[:], in_=bvr)
                nc.gpsimd.memset(onesrow[:], 1.0)

            # masks are ADDED to the scores psum via an extra matmul with
            # moving=identity: out[m,n] += st[n,m], so each stationary holds
            # the TRANSPOSE of the mask to apply in S^T [k(p), q(x)] coords.
            # diag tile mask M[k,q] = NEG where q < k -> st[q,k] = NEG where
            # p < x: keep where p - x >= 0.
            nc.gpsimd.memset(mt_diag[:], 0.0)
            nc.gpsimd.affine_select(
                out=mt_diag[:], in_=mt_diag[:], compare_op=mybir.AluOpType.is_ge,
                fill=NEG, base=0, channel_multiplier=1, pattern=[[-1, 128]])
            # off-4 tile mask M[k,q] = NEG where q > k -> st[q,k] = NEG where
            # p > x: keep where x - p >= 0.
            nc.gpsimd.memset(mt_off4[:], 0.0)
            nc.gpsimd.affine_select(
                out=mt_off4[:], in_=mt_off4[:], compare_op=mybir.AluOpType.is_ge,
                fill=NEG, base=0, channel_multiplier=-1, pattern=[[1, 128]])
            nc.gpsimd.memset(ones128[:], 1.0)
            nc.gpsimd.memset(ident[:], 1.0)
            nc.gpsimd.affine_select(
                out=ident[:], in_=ident[:], compare_op=mybir.AluOpType.is_equal,
                fill=0.0, base=0, channel_multiplier=-1, pattern=[[1, 128]])

            # ---- projections ----
            def rope_chunk(ps, dst, sl):
                """dst[:, sl] = rope(ps); ps is a psum view whose columns
                correspond to dst's slice sl. ScalarE stages the psum chunk
                to bf16 so the three DVE ops run in cheap 2x sbuf mode; the
                partition-half swap DMAs ride gpsimd's queue (sync's is full
                of input tiles). sint here is pre-swapped (see _rope_tables):
                out = tmp*cos + swap64(tmp)*sint."""
                tmp = upool.tile([128, 512], bf16, tag="u")
                tsw = upool.tile([128, 512], bf16, tag="ush")
                u2 = upool.tile([128, 512], bf16, tag="u2")
                nc.scalar.copy(tmp[:], ps)
                nc.gpsimd.dma_start(out=tsw[0:64, :], in_=tmp[64:128, :])
                nc.gpsimd.dma_start(out=tsw[64:128, :], in_=tmp[0:64, :])
                nc.vector.tensor_mul(dst[:, sl], tmp[:], cost[:, sl])
                nc.vector.tensor_mul(u2[:], tsw[:], sint[:, sl])
                nc.vector.tensor_add(dst[:, sl], dst[:, sl], u2[:])

            def bqs(lo, hi):
                return bq_t[:, lo:hi] if has_bias else None

            # P1: K and Q0 k-outer over full-width [128, S] psum tiles, so
            # matmuls chase the xT tile DMAs as they land. Two separate
            # pools so P2 (placed in K's banks) only waits on K's rope
            # drain, overlapping Q0's drain with P2 compute.
            with tc.tile_pool(name="projk", bufs=1, space="PSUM") as projk, \
                 tc.tile_pool(name="projq0", bufs=1, space="PSUM") as projq0:
                p1 = [
                    (lambda k: wk[:, k, :], bk_t, kt[:]),
                    (lambda k: wq[:, k, 0:128], bqs(0, 128), qt[:, 0, :]),
                ]
                tiles = [projk.tile([128, S], f32, tag="pjk", name="pj_k"),
                         projq0.tile([128, S], f32, tag="pjq", name="pj_q0")]
                for k in range(KTILES):
                    for (stf, btile, _), pt in zip(p1, tiles):
                        for n in range(S // 512):
                            nc.tensor.matmul(
                                pt[:, n * 512:(n + 1) * 512], stf(k),
                                xt[:, k, n * 512:(n + 1) * 512],
                                start=(k == 0),
                                stop=(k == KTILES - 1 and btile is None))
                if has_bias:
                    for (stf, btile, _), pt in zip(p1, tiles):
                        for n in range(S // 512):
                            nc.tensor.matmul(
                                pt[:, n * 512:(n + 1) * 512], btile,
                                onesrow[0:1, :], start=False, stop=True)
                for (_, _, dst), pt in zip(p1, tiles):
                    for n in range(S // 512):
                        sl = slice(n * 512, (n + 1) * 512)
                        rope_chunk(pt[:, sl], dst, sl)

            # P2: V^T, Q1-Q3 as 512-col chunks through a 4-deep psum pool;
            # chunk drains overlap the next chunk's accumulation. V^T->V
            # transposes run in their own 2-bank pool, spread across the Q1
            # chunk loop so they never stall the projection matmul stream.
            with tc.tile_pool(name="proj2", bufs=4, space="PSUM") as proj2, \
                 tc.tile_pool(name="tpp", bufs=2, space="PSUM") as tpp:

                def transpose_chunk(c):
                    for t in range(4):
                        sm = c * 4 + t
                        tp = tpp.tile([128, 128], bf16, tag="tp")
                        nc.tensor.transpose(
                            tp[:], vtsb[:, sm * 128:(sm + 1) * 128], ident[:])
                        nc.scalar.copy(vv[:, sm, :], tp[:])

                p2 = [
                    (lambda k: wq[:, k, 128:256], bqs(128, 256), qt[:, 1, :]),
                    (lambda k: wv[:, k, :], bv_t, None),
                    (lambda k: wq[:, k, 256:384], bqs(256, 384), qt[:, 2, :]),
                    (lambda k: wq[:, k, 384:512], bqs(384, 512), qt[:, 3, :]),
                ]
                todo_tp = []
                for ji, (stf, btile, dst) in enumerate(p2):
                    for c in range(S // 512):
                        sl = slice(c * 512, (c + 1) * 512)
                        pc = proj2.tile([128, 512], f32, tag="pc")
                        for k in range(KTILES):
                            nc.tensor.matmul(
                                pc[:], stf(k), xt[:, k, sl],
                                start=(k == 0),
                                stop=(k == KTILES - 1 and btile is None))
                        if has_bias:
                            nc.tensor.matmul(
                                pc[:], btile, onesrow[0:1, :],
                                start=False, stop=True)
                        if todo_tp:
                            transpose_chunk(todo_tp.pop(0))
                        if dst is None:
                            nc.scalar.copy(vtsb[:, sl], pc[:])
                            todo_tp.append(c)
                        else:
                            rope_chunk(pc[:], dst, sl)
                for c in todo_tp:
                    transpose_chunk(c)

            # ---- attention + (for last head) output projection ----
            with tc.tile_pool(name="score_psum", bufs=2, space="PSUM") as score_psum, \
                 tc.tile_pool(name="pv_psum", bufs=2, space="PSUM") as pv_psum, \
                 tc.tile_pool(name="out_psum", bufs=2, space="PSUM") as out_psum:

                def oproj(qi):
                    for ch in range(E // 512):
                        pso = out_psum.tile([128, 512], f32, tag="po2")
                        for h in range(GROUP):
                            nc.tensor.matmul(
                                pso[:], ot[:, h * ST + qi, :],
                                wo[:, h, ch * 512:(ch + 1) * 512],
                                start=(h == 0), stop=(h == GROUP - 1))
                        st = ostage.tile([128, 512], bf16, tag="st")
                        if ch % 2 == 0:
                            nc.vector.tensor_copy(st[:], pso[:])
                        else:
                            nc.scalar.copy(st[:], pso[:])
                        nc.sync.dma_start(
                            out=out[qi * 128:(qi + 1) * 128,
                                    ch * 512:(ch + 1) * 512],
                            in_=st[:])

                for m in range(GROUP):
                    e_tiles = {}
                    pair = {}

                    def pv_finish(qi):
                        """PV + rowsum for q-tile qi into half of a paired
                        [128,512] psum bank, laid out [pv_even | pv_odd |
                        rs_even | rs_odd] so that after the odd half ONE
                        contiguous [128,256] reciprocal + multiply normalize
                        both q-tiles at once."""
                        kjs = list(range(max(0, qi - WT), qi + 1))
                        even = (qi % 2 == 0)
                        if even:
                            pair['po'] = pv_psum.tile([128, 512], f32, tag="po",
                                                      name=f"po_{m}_{qi}")
                            pair['pv0'] = None
                        po = pair['po']
                        pvb = 0 if even else 128
                        rsb = 256 if even else 384
                        for j, kjj in enumerate(kjs):
                            off = (qi - kjj) * 128
                            mm = nc.tensor.matmul(
                                po[:, pvb:pvb + 128], vv[:, kjj, :],
                                e_tiles[kjj][:, off:off + 128],
                                start=(j == 0 and even),
                                stop=(j == len(kjs) - 1),
                                skip_group_check=not even)
                            if j == 0:
                                if even:
                                    pair['pv0'] = mm
                                else:
                                    # odd half relies on the even pv0's
                                    # start=True bank has_written clear
                                    add_dep_helper(mm.ins, pair['pv0'].ins,
                                                   sync=False,
                                                   reason="pair bank clear")
                        for j, kjj in enumerate(kjs):
                            off = (qi - kjj) * 128
                            mm = nc.tensor.matmul(
                                po[:, rsb:rsb + 128], ones128[:],
                                e_tiles[kjj][:, off:off + 128],
                                start=False, stop=(j == len(kjs) - 1),
                                skip_group_check=True)
                            if j == 0:
                                add_dep_helper(mm.ins, pair['pv0'].ins,
                                               sync=False,
                                               reason="rT after bank clear")
                        if not even:
                            rb = rbpool.tile([128, 256], f32, tag="rb")
                            nc.vector.reciprocal(rb[:], po[:, 256:512])
                            nc.vector.tensor_mul(
                                ot[:, m * ST + qi - 1:m * ST + qi + 1, :],
                                po[:, 0:256], rb[:])

                    for kj in range(ST):
                        nw = min(WT + 1, ST - kj)
                        W = 128 * nw
                        q0 = kj * 128
                        pss = score_psum.tile([128, 640], f32, tag="ss")
                        n0 = min(W, 512)
                        sa = nc.tensor.matmul(
                            pss[:, 0:n0], kt[:, q0:q0 + 128],
                            qt[:, m, q0:q0 + n0], start=True, stop=False)
                        ma = nc.tensor.matmul(
                            pss[:, 0:128], mt_diag[:], ident[:],
                            start=False, stop=True, skip_group_check=True)
                        add_dep_helper(ma.ins, sa.ins, sync=False,
                                       reason="mask add after score write")
                        if W > 512:
                            sb = nc.tensor.matmul(
                                pss[:, 512:W], kt[:, q0:q0 + 128],
                                qt[:, m, q0 + 512:q0 + W], start=True, stop=False)
                            mb = nc.tensor.matmul(
                                pss[:, 512:640], mt_off4[:], ident[:],
                                start=False, stop=True, skip_group_check=True)
                            add_dep_helper(mb.ins, sb.ins, sync=False,
                                           reason="mask add after score write")
                        e_t = epool.tile([128, 640], bf16, tag="e")
                        nc.scalar.activation(e_t[:, 0:W], pss[:, 0:W], Exp)
                        e_tiles[kj] = e_t
                        if kj >= 1:
                            pv_finish(kj - 1)
                            if m == GROUP - 1 and kj >= 3:
                                oproj(kj - 3)
                    pv_finish(ST - 1)
                    if m == GROUP - 1:
                        oproj(ST - 3)
                        oproj(ST - 2)
                        oproj(ST - 1)

    _split_sync_waits(nc)
    return nc


def _rope_tables():
    half = D // 2
    inv_freq = 1.0 / (ROPE_BASE ** (np.arange(half, dtype=np.float64) / half))
    ang = np.arange(S, dtype=np.float64)[:, None] * inv_freq[None, :]  # [S, 64]
    cos = np.cos(ang).T.astype(np.float32)          # [64, S]
    sin = np.sin(ang).T.astype(np.float32)
    cosT = np.concatenate([cos, cos], 0)            # [128, S]
    # sign-folded AND pre-swapped for the swap-first rotate-half formula
    # out = x*cos + swap64(x)*sinT
    sinT = np.concatenate([-sin, sin], 0)
    return np.ascontiguousarray(cosT), np.ascontiguousarray(sinT)


def kernel(x, Wq, bq, Wk, bk, Wv, bv, Wo, bo, **kw):
    x = np.asarray(x, np.float32)
    Wq = np.asarray(Wq, np.float32); bq = np.asarray(bq, np.float32)
    Wk = np.asarray(Wk, np.float32); bk = np.asarray(bk, np.float32)
    Wv = np.asarray(Wv, np.float32); bv = np.asarray(bv, np.float32)
    Wo = np.asarray(Wo, np.float32); bo = np.asarray(bo, np.float32)

    has_bias = bool(np.any(bq) or np.any(bk) or np.any(bv))
    nc = build_kernel(has_bias)

    bff = ml_dtypes.bfloat16
    cosT, sinT = _rope_tables()
    scale = 1.0 / np.sqrt(np.float32(D))

    in_maps = []
    for c in range(N_CORES):
        b, h = c // HKV, c % HKV
        qs = slice(h * HD_Q, (h + 1) * HD_Q)
        ks = slice(h * D, (h + 1) * D)
        m = {
            "xT": np.ascontiguousarray(x[b].T).astype(bff),
            "wqT": np.ascontiguousarray(Wq[qs].T).astype(bff),
            "wkT": np.ascontiguousarray((Wk[ks] * scale).T).astype(bff),
            "wvT": np.ascontiguousarray(Wv[ks].T).astype(bff),
            "woT": np.ascontiguousarray(Wo[:, qs].T).astype(bff),
            "cosT": cosT.astype(bff),
            "sinT": sinT.astype(bff),
        }
        if has_bias:
            m["bqr"] = np.ascontiguousarray(bq[qs][None, :]).astype(bff)
            m["bkr"] = np.ascontiguousarray((bk[ks] * scale)[None, :]).astype(bff)
            m["bvr"] = np.ascontiguousarray(bv[ks][None, :]).astype(bff)
        in_maps.append(m)

    res = run_bass_kernel_spmd(nc, in_maps, core_ids=list(range(N_CORES)))
    global LAST_RESULT
    LAST_RESULT = res
    if os.environ.get("BASS_KERNEL_RETIME"):
        # executable is now cached in-process: a second run times
        # transfer + device execution without compile.
        import time
        t0 = time.time()
        run_bass_kernel_spmd(nc, in_maps, core_ids=list(range(N_CORES)))
        print(f"retime run (transfer+exec): {time.time()-t0:.3f}s")

    out_full = np.zeros((B, S, E), np.float32)
    for c in range(N_CORES):
        out_full[c // HKV] += res.results[c]["out"].astype(np.float32)
    out_full += bo[None, None, :]
    return out_full


# revision 31
# speedup vs baseline: 1.0539x; 1.0539x over previous
"""GQA + sliding-window attention (B=2, S=2048, E=2048, HQ=16, HKV=4, D=128, W=512).

Sharding: 8 cores = 2 batches x 4 KV-head groups (tensor parallel).
Each core computes its batch's full sequence for one KV head + its 4 Q heads,
plus the (row-sharded) output projection partial; the host sums the 4 partials
per batch (the "all-reduce" done host-side) and adds bo.

v2 layout/schedule (vs v1):
  - k-outer projections accumulate K^T/Q^T/V^T into [128, S] PSUM tiles
    (4 banks each, 2 live), so matmuls start as soon as each xT k-tile's
    DMA lands instead of after the whole 14 MB input load.
  - V computed as V^T then turned into natural [s,d] via 16 PE transposes.
  - scores per k-tile into a double-buffered [128, 640] PSUM tile
    (512-col + 128-col matmuls, bank aligned), masked (DVE) + exp'd
    (ScalarE) without blocking the next k-tile's score matmul.
  - PV runs qi-major one tile behind exp, into a [128, 256] PSUM bank:
    cols 0:128 accumulate V^T E contributions, cols 128:256 accumulate
    the row-sum via an all-ones [k,128] stationary, which lands the
    softmax denominator replicated across all 128 partitions -- so
    normalization is just DVE reciprocal + multiply, no broadcast DMA.
  - output projection interleaved into the last head's loop; out is bf16.
"""

import os

import numpy as np
import ml_dtypes

import concourse.bass as bass
import concourse.mybir as mybir
import concourse.tile as tile
from concourse.tile import add_dep_helper
from concourse.bass_utils import run_bass_kernel_spmd

B, S, E = 2, 2048, 2048
HQ, HKV, D = 16, 4, 128
WINDOW = 512
ROPE_BASE = 10000.0
N_CORES = 8
GROUP = HQ // HKV          # 4 Q heads per KV head
HD_Q = GROUP * D           # 512
ST = S // 128              # 16 sequence tiles
KTILES = E // 128          # 16 contraction tiles over E
WT = WINDOW // 128         # 4 -> window spans WT+1 = 5 q-tiles
NEG = -30000.0

f32 = mybir.dt.float32
bf16 = mybir.dt.bfloat16


def _split_sync_waits(nc, max_waits=1):
    """walrus in this container rejects instructions with more than one
    sync-wait; split extras onto preceding same-engine NoOps."""
    for fn in nc.m.functions:
        for blk in fn.blocks:
            new_insts = []
            for inst in blk.instructions:
                si = getattr(inst, "sync_info", None)
                if si is not None and len(si.on_wait) > max_waits:
                    waits = list(si.on_wait)
                    head, tail = waits[:-max_waits], waits[-max_waits:]
                    for i in range(0, len(head), max_waits):
                        nop = mybir.InstNoOp(
                            name=f"splitwait-{nc.next_id()}",
                            ins=[], outs=[],
                            sync_info=mybir.SyncInfo(
                                on_wait=head[i:i + max_waits], on_update=[]),
                            bass_nofuse=True,
                        )
                        nop.engine = inst.engine
                        new_insts.append(nop)
                    inst.sync_info = mybir.SyncInfo(
                        on_wait=tail, on_update=list(si.on_update))
                new_insts.append(inst)
            blk.instructions[:] = new_insts


def _scalar_recip(nc, out, in_):
    """ScalarE Reciprocal activation. bass bans it for precision-sensitive
    uses (~1e-5 rel err); that is far inside this kernel's error budget and
    it frees the (saturated) DVE from 64 slow InstReciprocals."""
    sc = nc.scalar
    inputs = [sc.lower_ap(in_)]
    for arg in (0.0, 1.0, 0.0):  # bias, scale, alpha immediates
        inputs.append(mybir.ImmediateValue(dtype=mybir.dt.float32, value=arg))
    return sc.add_instruction(mybir.InstActivation(
        name=sc.bass.get_next_instruction_name(),
        func=mybir.ActivationFunctionType.Reciprocal,
        ins=inputs, outs=[sc.lower_ap(out)]))


def build_kernel(has_bias):
    nc = bass.Bass("TRN2", target_bir_lowering=False, debug=False,
                   num_devices=N_CORES)
    Exp = mybir.ActivationFunctionType.Exp

    xT = nc.dram_tensor("xT", [E, S], bf16, kind="ExternalInput").ap()
    wqT = nc.dram_tensor("wqT", [E, HD_Q], bf16, kind="ExternalInput").ap()
    wkT = nc.dram_tensor("wkT", [E, D], bf16, kind="ExternalInput").ap()
    wvT = nc.dram_tensor("wvT", [E, D], bf16, kind="ExternalInput").ap()
    woT = nc.dram_tensor("woT", [HD_Q, E], bf16, kind="ExternalInput").ap()
    cosT = nc.dram_tensor("cosT", [D, S], bf16, kind="ExternalInput").ap()
    sinT = nc.dram_tensor("sinT", [D, S], bf16, kind="ExternalInput").ap()
    if has_bias:
        bqr = nc.dram_tensor("bqr", [1, HD_Q], bf16, kind="ExternalInput").ap()
        bkr = nc.dram_tensor("bkr", [1, D], bf16, kind="ExternalInput").ap()
        bvr = nc.dram_tensor("bvr", [1, D], bf16, kind="ExternalInput").ap()
    out = nc.dram_tensor("out", [S, E], bf16, kind="ExternalOutput").ap()

    with tile.TileContext(nc) as tc:
        with tc.tile_pool(name="singles", bufs=1) as singles, \
             tc.tile_pool(name="upool", bufs=6) as upool, \
             tc.tile_pool(name="epool", bufs=7) as epool, \
             tc.tile_pool(name="rbpool", bufs=3) as rbpool, \
             tc.tile_pool(name="ostage", bufs=4) as ostage:

            # ---- resident tensors ----
            xt = singles.tile([128, KTILES, S], bf16)
            wq = singles.tile([128, KTILES, HD_Q], bf16)
            wk = singles.tile([128, KTILES, D], bf16)
            wv = singles.tile([128, KTILES, D], bf16)
            wo = singles.tile([128, GROUP, E], bf16)
            cost = singles.tile([128, S], bf16)
            sint = singles.tile([128, S], bf16)
            qt = singles.tile([128, GROUP, S], bf16)
            kt = singles.tile([128, S], bf16)
            vtsb = singles.tile([128, S], bf16)
            vv = singles.tile([128, ST, D], bf16)
            ot = singles.tile([128, GROUP * ST, D], bf16)
            mt_diag = singles.tile([128, 128], bf16)
            mt_off4 = singles.tile([128, 128], bf16)
            ones128 = singles.tile([128, 128], bf16)
            ident = singles.tile([128, 128], bf16)

            # Input loads. Descriptor ISSUE on one queue is ~650ns each, so
            # bulk weights go as single rearranged descriptors, and the two
            # tensors P1 needs at k=0 (wk, wq head 0) issue on ScalarE's DMA
            # queue so they don't queue behind the xT tile stream on sync's.
            def ptd(t):  # [(t p) d] dram layout -> [p t d]
                return t.rearrange("(t p) d -> p t d", p=128)

            nc.scalar.dma_start(out=wk[:], in_=ptd(wkT))
            nc.scalar.dma_start(out=wq[:, :, 0:128], in_=ptd(wqT[:, 0:128]))
            for t in range(KTILES):
                nc.sync.dma_start(out=xt[:, t, :], in_=xT[t * 128:(t + 1) * 128, :])
            nc.sync.dma_start(out=sint[:], in_=sinT)
            nc.sync.dma_start(out=cost[:], in_=cosT)
            nc.sync.dma_start(out=wq[:, :, 128:256], in_=ptd(wqT[:, 128:256]))
            nc.sync.dma_start(out=wv[:], in_=ptd(wvT))
            nc.sync.dma_start(out=wq[:, :, 256:384], in_=ptd(wqT[:, 256:384]))
            nc.sync.dma_start(out=wq[:, :, 384:512], in_=ptd(wqT[:, 384:512]))
            nc.sync.dma_start(out=wo[:], in_=woT.rearrange("(h p) e -> p h e", p=128))
            bq_t = bk_t = bv_t = onesrow = None
            if has_bias:
                bq_t = singles.tile([1, HD_Q], bf16)
                bk_t = singles.tile([1, D], bf16)
                bv_t = singles.tile([1, D], bf16)
                onesrow = singles.tile([1, 512], bf16)
                nc.sync.dma_start(out=bq_t[:], in_=bqr)
                nc.sync.dma_start(out=bk_t[:], in_=bkr)
                nc.sync.dma_start(out=bv_t[:], in_=bvr)
                nc.gpsimd.memset(onesrow[:], 1.0)

            # masks are ADDED to the scores psum via an extra matmul with
            # moving=identity: out[m,n] += st[n,m], so each stationary holds
            # the TRANSPOSE of the mask to apply in S^T [k(p), q(x)] coords.
            # diag tile mask M[k,q] = NEG where q < k -> st[q,k] = NEG where
            # p < x: keep where p - x >= 0.
            nc.gpsimd.memset(mt_diag[:], 0.0)
            nc.gpsimd.affine_select(
                out=mt_diag[:], in_=mt_diag[:], compare_op=mybir.AluOpType.is_ge,
                fill=NEG, base=0, channel_multiplier=1, pattern=[[-1, 128]])
            # off-4 tile mask M[k,q] = NEG where q > k -> st[q,k] = NEG where
            # p > x: keep where x - p >= 0.
            nc.gpsimd.memset(mt_off4[:], 0.0)
            nc.gpsimd.affine_select(
                out=mt_off4[:], in_=mt_off4[:], compare_op=mybir.AluOpType.is_ge,
                fill=NEG, base=0, channel_multiplier=-1, pattern=[[1, 128]])
            nc.gpsimd.memset(ones128[:], 1.0)
            nc.gpsimd.memset(ident[:], 1.0)
            nc.gpsimd.affine_select(
                out=ident[:], in_=ident[:], compare_op=mybir.AluOpType.is_equal,
                fill=0.0, base=0, channel_multiplier=-1, pattern=[[1, 128]])

            # ---- projections ----
            def rope_chunk(ps, dst, sl):
                """dst[:, sl] = rope(ps); ps is a psum view whose columns
                correspond to dst's slice sl. ScalarE stages the psum chunk
                to bf16 so the three DVE ops run in cheap 2x sbuf mode; the
                partition-half swap DMAs ride gpsimd's queue (sync's is full
                of input tiles). sint here is pre-swapped (see _rope_tables):
                out = tmp*cos + swap64(tmp)*sint."""
                tmp = upool.tile([128, 512], bf16, tag="u")
                tsw = upool.tile([128, 512], bf16, tag="ush")
                u2 = upool.tile([128, 512], bf16, tag="u2")
                nc.scalar.copy(tmp[:], ps)
                nc.gpsimd.dma_start(out=tsw[0:64, :], in_=tmp[64:128, :])
                nc.gpsimd.dma_start(out=tsw[64:128, :], in_=tmp[0:64, :])
                nc.vector.tensor_mul(dst[:, sl], tmp[:], cost[:, sl])
                nc.vector.tensor_mul(u2[:], tsw[:], sint[:, sl])
                nc.vector.tensor_add(dst[:, sl], dst[:, sl], u2[:])

            def bqs(lo, hi):
                return bq_t[:, lo:hi] if has_bias else None

            # P1: K and Q0 k-outer over full-width [128, S] psum tiles, so
            # matmuls chase the xT tile DMAs as they land. Two separate
            # pools so P2 (placed in K's banks) only waits on K's rope
            # drain, overlapping Q0's drain with P2 compute.
            with tc.tile_pool(name="projk", bufs=1, space="PSUM") as projk, \
                 tc.tile_pool(name="projq0", bufs=1, space="PSUM") as projq0:
                p1 = [
                    (lambda k: wk[:, k, :], bk_t, kt[:]),
                    (lambda k: wq[:, k, 0:128], bqs(0, 128), qt[:, 0, :]),
                ]
                tiles = [projk.tile([128, S], f32, tag="pjk", name="pj_k"),
                         projq0.tile([128, S], f32, tag="pjq", name="pj_q0")]
                for k in range(KTILES):
                    for (stf, btile, _), pt in zip(p1, tiles):
                        for n in range(S // 512):
                            nc.tensor.matmul(
                                pt[:, n * 512:(n + 1) * 512], stf(k),
                                xt[:, k, n * 512:(n + 1) * 512],
                                start=(k == 0),
                                stop=(k == KTILES - 1 and btile is None))
                if has_bias:
                    for (stf, btile, _), pt in zip(p1, tiles):
                        for n in range(S // 512):
                            nc.tensor.matmul(
                                pt[:, n * 512:(n + 1) * 512], btile,
                                onesrow[0:1, :], start=False, stop=True)
                for (_, _, dst), pt in zip(p1, tiles):
                    for n in range(S // 512):
                        sl = slice(n * 512, (n + 1) * 512)
                        rope_chunk(pt[:, sl], dst, sl)

            # P2: V^T, Q1-Q3 as 512-col chunks through a 4-deep psum pool;
            # chunk drains overlap the next chunk's accumulation. V^T->V
            # transposes run in their own 2-bank pool, spread across the Q1
            # chunk loop so they never stall the projection matmul stream.
            with tc.tile_pool(name="proj2", bufs=4, space="PSUM") as proj2, \
                 tc.tile_pool(name="tpp", bufs=2, space="PSUM") as tpp:

                def transpose_chunk(c):
                    for t in range(4):
                        sm = c * 4 + t
                        tp = tpp.tile([128, 128], bf16, tag="tp")
                        nc.tensor.transpose(
                            tp[:], vtsb[:, sm * 128:(sm + 1) * 128], ident[:])
                        nc.scalar.copy(vv[:, sm, :], tp[:])

                p2 = [
                    (lambda k: wq[:, k, 128:256], bqs(128, 256), qt[:, 1, :]),
                    (lambda k: wv[:, k, :], bv_t, None),
                    (lambda k: wq[:, k, 256:384], bqs(256, 384), qt[:, 2, :]),
                    (lambda k: wq[:, k, 384:512], bqs(384, 512), qt[:, 3, :]),
                ]
                todo_tp = []
                for ji, (stf, btile, dst) in enumerate(p2):
                    for c in range(S // 512):
                        sl = slice(c * 512, (c + 1) * 512)
                        pc = proj2.tile([128, 512], f32, tag="pc")
                        for k in range(KTILES):
                            nc.tensor.matmul(
                                pc[:], stf(k), xt[:, k, sl],
                                start=(k == 0),
                                stop=(k == KTILES - 1 and btile is None))
                        if has_bias:
                            nc.tensor.matmul(
                                pc[:], btile, onesrow[0:1, :],
                                start=False, stop=True)
                        if todo_tp:
                            transpose_chunk(todo_tp.pop(0))
                        if dst is None:
                            nc.scalar.copy(vtsb[:, sl], pc[:])
                            todo_tp.append(c)
                        else:
                            rope_chunk(pc[:], dst, sl)
                for c in todo_tp:
                    transpose_chunk(c)

            # ---- attention + (for last head) output projection ----
            with tc.tile_pool(name="score_psum", bufs=2, space="PSUM") as score_psum, \
                 tc.tile_pool(name="pv_psum", bufs=2, space="PSUM") as pv_psum, \
                 tc.tile_pool(name="out_psum", bufs=2, space="PSUM") as out_psum:

                def oproj(qi):
                    for ch in range(E // 512):
                        pso = out_psum.tile([128, 512], f32, tag="po2")
                        for h in range(GROUP):
                            nc.tensor.matmul(
                                pso[:], ot[:, h * ST + qi, :],
                                wo[:, h, ch * 512:(ch + 1) * 512],
                                start=(h == 0), stop=(h == GROUP - 1))
                        st = ostage.tile([128, 512], bf16, tag="st")
                        if ch % 2 == 0:
                            nc.vector.tensor_copy(st[:], pso[:])
                        else:
                            nc.scalar.copy(st[:], pso[:])
                        nc.sync.dma_start(
                            out=out[qi * 128:(qi + 1) * 128,
                                    ch * 512:(ch + 1) * 512],
                            in_=st[:])

                for m in range(GROUP):
                    e_tiles = {}
                    pair = {}

                    def pv_finish(qi):
                        """PV + rowsum for q-tile qi into half of a paired
                        [128,512] psum bank, laid out [pv_even | pv_odd |
                        rs_even | rs_odd] so that after the odd half ONE
                        contiguous [128,256] reciprocal + multiply normalize
                        both q-tiles at once."""
                        kjs = list(range(max(0, qi - WT), qi + 1))
                        even = (qi % 2 == 0)
                        if even:
                            pair['po'] = pv_psum.tile([128, 512], f32, tag="po",
                                                      name=f"po_{m}_{qi}")
                            pair['pv0'] = None
                        po = pair['po']
                        pvb = 0 if even else 128
                        rsb = 256 if even else 384
                        for j, kjj in enumerate(kjs):
                            off = (qi - kjj) * 128
                            mm = nc.tensor.matmul(
                                po[:, pvb:pvb + 128], vv[:, kjj, :],
                                e_tiles[kjj][:, off:off + 128],
                                start=(j == 0 and even),
                                stop=(j == len(kjs) - 1),
                                skip_group_check=not even)
                            if j == 0:
                                if even:
                                    pair['pv0'] = mm
                                else:
                                    # odd half relies on the even pv0's
                                    # start=True bank has_written clear
                                    add_dep_helper(mm.ins, pair['pv0'].ins,
                                                   sync=False,
                                                   reason="pair bank clear")
                        for j, kjj in enumerate(kjs):
                            off = (qi - kjj) * 128
                            mm = nc.tensor.matmul(
                                po[:, rsb:rsb + 128], ones128[:],
                                e_tiles[kjj][:, off:off + 128],
                                start=False, stop=(j == len(kjs) - 1),
                                skip_group_check=True)
                            if j == 0:
                                add_dep_helper(mm.ins, pair['pv0'].ins,
                                               sync=False,
                                               reason="rT after bank clear")
                        if not even:
                            rb = rbpool.tile([128, 256], f32, tag="rb")
                            nc.vector.reciprocal(rb[:], po[:, 256:512])
                            nc.vector.tensor_mul(
                                ot[:, m * ST + qi - 1:m * ST + qi + 1, :],
                                po[:, 0:256], rb[:])

                    for kj in range(ST):
                        nw = min(WT + 1, ST - kj)
                        W = 128 * nw
                        q0 = kj * 128
                        pss = score_psum.tile([128, 640], f32, tag="ss")
                        n0 = min(W, 512)
                        sa = nc.tensor.matmul(
                            pss[:, 0:n0], kt[:, q0:q0 + 128],
                            qt[:, m, q0:q0 + n0], start=True, stop=False)
                        ma = nc.tensor.matmul(
                            pss[:, 0:128], mt_diag[:], ident[:],
                            start=False, stop=True, skip_group_check=True)
                        add_dep_helper(ma.ins, sa.ins, sync=False,
                                       reason="mask add after score write")
                        if W > 512:
                            sb = nc.tensor.matmul(
                                pss[:, 512:W], kt[:, q0:q0 + 128],
                                qt[:, m, q0 + 512:q0 + W], start=True, stop=False)
                            mb = nc.tensor.matmul(
                                pss[:, 512:640], mt_off4[:], ident[:],
                                start=False, stop=True, skip_group_check=True)
                            add_dep_helper(mb.ins, sb.ins, sync=False,
                                           reason="mask add after score write")
                        e_t = epool.tile([128, 640], bf16, tag="e")
                        nc.scalar.activation(e_t[:, 0:W], pss[:, 0:W], Exp)
                        e_tiles[kj] = e_t
                        if kj >= 1:
                            pv_finish(kj - 1)
                            if m == GROUP - 1 and kj >= 3:
                                oproj(kj - 3)
                    pv_finish(ST - 1)
                    if m == GROUP - 1:
                        oproj(ST - 3)
                        oproj(ST - 2)
                        oproj(ST - 1)

    _split_sync_waits(nc)
    return nc


def _rope_tables():
    half = D // 2
    inv_freq = 1.0 / (ROPE_BASE ** (np.arange(half, dtype=np.float64) / half))
    ang = np.arange(S, dtype=np.float64)[:, None] * inv_freq[None, :]  # [S, 64]
    cos = np.cos(ang).T.astype(np.float32)          # [64, S]
    sin = np.sin(ang).T.astype(np.float32)
    cosT = np.concatenate([cos, cos], 0)            # [128, S]
    # sign-folded AND pre-swapped for the swap-first rotate-half formula
    # out = x*cos + swap64(x)*sinT
    sinT = np.concatenate([-sin, sin], 0)
    return np.ascontiguousarray(cosT), np.ascontiguousarray(sinT)


def kernel(x, Wq, bq, Wk, bk, Wv, bv, Wo, bo, **kw):
    x = np.asarray(x, np.float32)
    Wq = np.asarray(Wq, np.float32); bq = np.asarray(bq, np.float32)
    Wk = np.asarray(Wk, np.float32); bk = np.asarray(bk, np.float32)
    Wv = np.asarray(Wv, np.float32); bv = np.asarray(bv, np.float32)
    Wo = np.asarray(Wo, np.float32); bo = np.asarray(bo, np.float32)

    has_bias = bool(np.any(bq) or np.any(bk) or np.any(bv))
    nc = build_kernel(has_bias)

    bff = ml_dtypes.bfloat16
    cosT, sinT = _rope_tables()
    scale = 1.0 / np.sqrt(np.float32(D))

    in_maps = []
    for c in range(N_CORES):
        b, h = c // HKV, c % HKV
        qs = slice(h * HD_Q, (h + 1) * HD_Q)
        ks = slice(h * D, (h + 1) * D)
        m = {
            "xT": np.ascontiguousarray(x[b].T).astype(bff),
            "wqT": np.ascontiguousarray(Wq[qs].T).astype(bff),
            "wkT": np.ascontiguousarray((Wk[ks] * scale).T).astype(bff),
            "wvT": np.ascontiguousarray(Wv[ks].T).astype(bff),
            "woT": np.ascontiguousarray(Wo[:, qs].T).astype(bff),
            "cosT": cosT.astype(bff),
            "sinT": sinT.astype(bff),
        }
        if has_bias:
            m["bqr"] = np.ascontiguousarray(bq[qs][None, :]).astype(bff)
            m["bkr"] = np.ascontiguousarray((bk[ks] * scale)[None, :]).astype(bff)
            m["bvr"] = np.ascontiguousarray(bv[ks][None, :]).astype(bff)
        in_maps.append(m)

    res = run_bass_kernel_spmd(nc, in_maps, core_ids=list(range(N_CORES)))
    global LAST_RESULT
    LAST_RESULT = res
    if os.environ.get("BASS_KERNEL_RETIME"):
        # executable is now cached in-process: a second run times
        # transfer + device execution without compile.
        import time
        t0 = time.time()
        run_bass_kernel_spmd(nc, in_maps, core_ids=list(range(N_CORES)))
        print(f"retime run (transfer+exec): {time.time()-t0:.3f}s")

    out_full = np.zeros((B, S, E), np.float32)
    for c in range(N_CORES):
        out_full[c // HKV] += res.results[c]["out"].astype(np.float32)
    out_full += bo[None, None, :]
    return out_full
